# revision 23
# baseline (speedup 1.0000x reference)
"""Trainium2 Bass kernel for nn_AttentionBlock (B=4, S=2048, D=1024, single head).

Sharding: 8 cores = 4 batches x 2 query-halves; each core owns 1024 queries
of one batch and returns that [1024, 1024] slice of the output (transposed;
the host gather transposes it back).

Algebraic restructure with two weight-only folds (host-side, x-independent):
    W2 = Wk^T @ Wq   ->  scores = Q K^T = Xq Wq^T Wk Xk^T = (X W2 Xq^T)^T
    W3 = Wp @ Wv     ->  attn V Wp^T = attn (X W3^T)
so per core:
    G  [D, 1024]    = W2 @ Xq^T
    scoresT / VP    = fused loop over 16 key tiles: each streamed 128x128
                      X^T stationary tile feeds 4 matmuls (2 scoresT chunks,
                      2 VP chunks); VP = X @ W3^T stays resident in SBUF
    expT = exp(scoresT * scale)   (no max subtraction; scores are O(1))
    rowsum r_row via ones-column matmul; r broadcast to [128, SQ] via a
    K=1 ones-row matmul (fp32)
    yT [D, 1024] = VP.T @ expT in two f-group passes (4 f-tiles = all PSUM),
    then yT * r_bcast + bias_col on VectorE, DMA out transposed.
Q, K, V, and the output projection never exist on the device.

Matmuls run in float32r (fp32 storage, reduced-precision PE multiply,
1 cycle/row vs fp32's 4). Keys are permuted per-core (own half first) -
attention is permutation invariant.

SBUF tags are aliased across phases (pool memory is the static sum over
tags): xq0..7 carry Xq^T -> VP[8..15]; g0..7 carry G -> r_row/r_bcast;
expT8..15 carry W2^T before scores reach them; w0..7 carry W3^T.
PSUM uses 4 double-bank tags q0..q3.
"""

import numpy as np
from contextlib import ExitStack

D = 1024
S = 2048
SQ = 1024  # queries per core
P = 128
SCALE = float(1.0 / np.sqrt(np.float32(D)).astype(np.float32))

_CACHED = {}


def _build_nc():
    import concourse.bass as bass
    import concourse.tile as tile
    from concourse import bacc, mybir

    DT = mybir.dt.float32
    F32R = mybir.dt.float32r
    FP = mybir.dt.float32
    Exp = mybir.ActivationFunctionType.Exp
    MUL = mybir.AluOpType.mult
    ADD = mybir.AluOpType.add

    nc = bacc.Bacc("TRN2", target_bir_lowering=False)
    xt_d = nc.declare_dram_parameter("xt", [D, S], F32R, isOutput=False)
    w2t_d = nc.declare_dram_parameter("w2t", [D, D], F32R, isOutput=False)
    w3t_d = nc.declare_dram_parameter("w3t", [D, D], F32R, isOutput=False)
    biasc_d = nc.declare_dram_parameter("biasc", [P, 8], DT, isOutput=False)
    ones_d = nc.declare_dram_parameter("ones", [P, 1], F32R, isOutput=False)
    onesr_d = nc.declare_dram_parameter("onesr", [1, P], DT, isOutput=False)
    yt_d = nc.declare_dram_parameter("yt", [D, SQ], DT, isOutput=True)

    ND = D // P     # 8 tiles along D
    NS = S // P     # 16 tiles along S

    with tile.TileContext(nc) as tc:
        with ExitStack() as ctx:
            pool = ctx.enter_context(tc.tile_pool(name="main", bufs=1))
            psum = ctx.enter_context(tc.tile_pool(name="psum", bufs=1, space="PSUM"))

            def ptile(shape, name, tag, bufs=1, dt=F32R):
                return pool.tile(shape, dt, name=name, tag=tag, bufs=bufs)

            def qbank(i, name, shape=(P, 1024)):
                # 4 PSUM tags x 2 banks each = all 8 banks
                return psum.tile(list(shape), FP, name=name, tag=f"q{i}", bufs=1)

            # ---- resident inputs: W2^T + Xq^T first (phase-1 critical path)
            xq = []
            w2t = []
            for d in range(ND):
                t = ptile([P, D], f"w2t{d}", f"expT{8 + d}")
                nc.sync.dma_start(t[:], w2t_d[d * P:(d + 1) * P, :])
                w2t.append(t)
                t = ptile([P, SQ], f"xq{d}", f"xq{d}")
                nc.sync.dma_start(t[:], xt_d[d * P:(d + 1) * P, 0:SQ])
                xq.append(t)
            w3t = []
            for d in range(ND):
                t = ptile([P, D], f"w{d}", f"w{d}")
                nc.sync.dma_start(t[:], w3t_d[d * P:(d + 1) * P, :])
                w3t.append(t)
            ones_sb = ptile([P, 1], "ones", "ones")
            nc.sync.dma_start(ones_sb[:], ones_d[:, :])
            onesr_sb = ptile([1, P], "onesr", "onesr", dt=DT)
            nc.sync.dma_start(onesr_sb[:], onesr_d[:, :])
            biasc_sb = ptile([P, 8], "biasc", "biasc", dt=DT)
            nc.sync.dma_start(biasc_sb[:], biasc_d[:, :])

            # ---- phase 1: G[g][128, SQ] = sum_d w2t[d][:, g].T @ xq[d] ----
            g_sb = []
            for g in range(ND):
                g_sb.append(ptile([P, SQ], f"g{g}", f"g{g}"))
            for g in range(ND):
                pg = qbank(g % 4, f"pg_{g}")
                ps0, ps1 = pg[:, 0:512], pg[:, 512:1024]
                for d in range(ND):
                    lt = w2t[d][:, g * P:(g + 1) * P]
                    nc.tensor.matmul(ps0, lt, xq[d][:, 0:512],
                                     start=(d == 0), stop=(d == ND - 1))
                    nc.tensor.matmul(ps1, lt, xq[d][:, 512:1024],
                                     start=(d == 0), stop=(d == ND - 1))
                nc.vector.tensor_copy(g_sb[g][:, 0:512], ps0)
                nc.vector.tensor_copy(g_sb[g][:, 512:1024], ps1)

            # ---- phase 2 (fused): per key tile sk, stream the X^T stationary
            #      tile once; 2 matmuls for scoresT (-> exp -> expT) and 2 for
            #      VP = X @ W3^T. VP[sk] stays resident in SBUF. ----
            expT = []
            for sk in range(NS):
                expT.append(ptile([P, SQ], f"expT{sk}", f"expT{sk}"))
            vp = []
            for sk in range(NS):
                tag = f"vres{sk}" if sk < 8 else f"xq{sk - 8}"
                vp.append(ptile([P, D], f"vp{sk}", tag))
            for sk in range(NS):
                psc = qbank(sk % 2, f"psc_{sk}")
                ps0, ps1 = psc[:, 0:512], psc[:, 512:1024]
                pv = qbank(2 + sk % 2, f"pv_{sk}")
                pv0, pv1 = pv[:, 0:512], pv[:, 512:1024]
                for d in range(ND):
                    xs = ptile([P, P], f"xs_{sk}_{d}", "xs", bufs=12)
                    nc.sync.dma_start(
                        xs[:], xt_d[d * P:(d + 1) * P, sk * P:(sk + 1) * P])
                    nc.tensor.matmul(ps0, xs[:], g_sb[d][:, 0:512],
                                     start=(d == 0), stop=(d == ND - 1))
                    nc.tensor.matmul(ps1, xs[:], g_sb[d][:, 512:1024],
                                     start=(d == 0), stop=(d == ND - 1))
                    nc.tensor.matmul(pv0, xs[:], w3t[d][:, 0:512],
                                     start=(d == 0), stop=(d == ND - 1))
                    nc.tensor.matmul(pv1, xs[:], w3t[d][:, 512:1024],
                                     start=(d == 0), stop=(d == ND - 1))
                nc.scalar.activation(expT[sk][:, 0:512], ps0, Exp, scale=SCALE)
                nc.scalar.activation(expT[sk][:, 512:1024], ps1, Exp, scale=SCALE)
                nc.vector.tensor_copy(vp[sk][:, 0:512], pv0)
                nc.vector.tensor_copy(vp[sk][:, 512:1024], pv1)

            # ---- phase 3: rowsum -> r_bcast [128, SQ] ----
            pc = qbank(0, "pcs", shape=(1, 1024))
            pc0, pc1 = pc[0:1, 0:512], pc[0:1, 512:1024]
            for sk in range(NS):
                nc.tensor.matmul(pc0, ones_sb[:], expT[sk][:, 0:512],
                                 start=(sk == 0), stop=(sk == NS - 1))
                nc.tensor.matmul(pc1, ones_sb[:], expT[sk][:, 512:1024],
                                 start=(sk == 0), stop=(sk == NS - 1))
            r_row = ptile([1, SQ], "r_row", "g0", dt=DT)
            nc.vector.reciprocal(r_row[0:1, 0:512], pc0)
            nc.vector.reciprocal(r_row[0:1, 512:1024], pc1)
            # broadcast r_row across partitions via K=1 fp32 matmul
            prb = qbank(1, "prb")
            nc.tensor.matmul(prb[:, 0:512], onesr_sb[:], r_row[0:1, 0:512],
                             start=True, stop=True)
            nc.tensor.matmul(prb[:, 512:1024], onesr_sb[:],
                             r_row[0:1, 512:1024], start=True, stop=True)
            rb_sb = ptile([P, SQ], "rb_sb", "g1", dt=DT)
            nc.vector.tensor_copy(rb_sb[:, 0:512], prb[:, 0:512])
            nc.vector.tensor_copy(rb_sb[:, 512:1024], prb[:, 512:1024])

            # ---- phase 4: yT[f][128, SQ] = sum_sk VP[sk][:, f].T @ expT[sk],
            #      two f-group passes; normalize + bias on VectorE; DMA out ----
            for fg in range(2):
                otp = [qbank(i, f"ot_{fg}_{i}") for i in range(4)]
                for sk in range(NS):
                    for i in range(4):
                        f = fg * 4 + i
                        lt = vp[sk][:, f * P:(f + 1) * P]
                        nc.tensor.matmul(otp[i][:, 0:512], lt,
                                         expT[sk][:, 0:512],
                                         start=(sk == 0), stop=(sk == NS - 1))
                        nc.tensor.matmul(otp[i][:, 512:1024], lt,
                                         expT[sk][:, 512:1024],
                                         start=(sk == 0), stop=(sk == NS - 1))
                for i in range(4):
                    f = fg * 4 + i
                    ysb = ptile([P, SQ], f"ysb_{f}", f"g{2 + f % 4}", dt=DT)
                    nc.vector.tensor_tensor(ysb[:], otp[i][:], rb_sb[:], MUL)
                    nc.vector.tensor_scalar_add(ysb[:], ysb[:],
                                                biasc_sb[:, f:f + 1])
                    nc.sync.dma_start(yt_d[f * P:(f + 1) * P, :], ysb[:])

    nc.compile()
    return nc


def _get_nc():
    if "nc" not in _CACHED:
        _CACHED["nc"] = _build_nc()
    return _CACHED["nc"]


def make_in_maps(x, w_qkv, w_proj, b_proj):
    wq = w_qkv[0:D]
    wk = w_qkv[D:2 * D]
    wv = w_qkv[2 * D:3 * D]
    w2 = wk.T @ wq                   # scores = X W2 Xq^T
    w3 = w_proj @ wv                 # attn V Wp^T = attn (X W3^T)
    w2T = np.ascontiguousarray(w2.T)
    w3T = np.ascontiguousarray(w3.T)
    biasc = np.ascontiguousarray(b_proj.reshape(8, P).T)
    ones = np.ones((P, 1), dtype=np.float32)
    onesr = np.ones((1, P), dtype=np.float32)
    in_maps = []
    for c in range(8):
        b, h = c // 2, c % 2
        own = x[b, h * SQ:(h + 1) * SQ]       # [1024, D] our queries
        other = x[b, (1 - h) * SQ:(2 - h) * SQ]
        xt = np.ascontiguousarray(np.concatenate([own.T, other.T], axis=1))
        in_maps.append({
            "xt": xt, "w2t": w2T, "w3t": w3T,
            "biasc": biasc, "ones": ones, "onesr": onesr,
        })
    return in_maps


def gather_out(results):
    out = np.empty((4, S, D), dtype=np.float32)
    for c in range(8):
        b, h = c // 2, c % 2
        out[b, h * SQ:(h + 1) * SQ] = results[c]["yt"].T
    return out


def kernel(x, w_qkv, w_proj, b_proj):
    from concourse import bass_utils
    nc = _get_nc()
    in_maps = make_in_maps(np.asarray(x, dtype=np.float32),
                           np.asarray(w_qkv, dtype=np.float32),
                           np.asarray(w_proj, dtype=np.float32),
                           np.asarray(b_proj, dtype=np.float32))
    res = bass_utils.run_bass_kernel_spmd(nc, in_maps, list(range(8))).results
    return gather_out(res)
